# revision 18
# baseline (speedup 1.0000x reference)
"""AttentionRPE kernel for 8 Trainium2 NeuronCores — bf16 single-pass design.

Math (per (b,s) row, T=128 targets, D=256, H=8 heads, DH=32, DR=32):
  q   = src @ Wsrc.T + bsrc                       [D]
  K'  = tgt @ Wk.T + rpe @ Rwk.T                  [T, D]
  att = softmax_h(q_h . K'_h / sqrt(DH))          [H, T]   (masked)
  out = (att @ V')_heads @ Wout.T + bout          [D]

Device formulation:
  * The whole q-path is linear in src, so qw = (q/sqrt(DH)) @ Wkx is folded
    ON HOST into per-row stationary vectors qk[(s,h), f] (f = 288 tgt|rpe
    features).  logits[(s,h), t] = sum_f qk[f,(s,h)] * tgtxT[f, t].
  * Host ships tgtx in BOTH layouts as bf16: transposed (for the logits
    moving operand) and natural (stationary for the G path).  No on-chip
    transposes of the big tensor; all matmuls are single-pass bf16 (FWL
    fast-weight-load kicks in on the 128-col stationaries).
  * Padding mask + off-window garbage masking folded into the 3rd logits
    matmul as 4 extra stationary rows (per-group one-hot selector).
  * G[f, j, h] = sum_t tgtx[t,f] * att[h,t] via natural-tgtx stationary +
    transposed-attention moving (8 cols/row).  Final: out = sum_k
    gall[:,k,:].T @ wfx[k] with host-folded wfx = (Wout_h @ Wvx_h).T.
  * All per-block data rides in 2 large DMAs (one per HWDGE engine) —
    HWDGE dispatch is ~0.7us/call on the issuing engine queue, so DMA
    count is minimized.

Sharding: 1024 (b,s) rows split contiguously over 8 cores (128 each).
"""

import numpy as np
import ml_dtypes

import concourse.bass as bass
import concourse.bacc as bacc
import concourse.mybir as mybir
from concourse.tile import TileContext
from concourse.masks import make_identity
from concourse.bass_utils import run_bass_kernel_spmd

B, S, T, D = 2, 512, 128, 256
H, DH, DR = 8, 32, 32
DX = D + DR          # 288 = tgt|rpe feature dim
DOUT = D
NCORES = 8
BS = B * S           # 1024 total rows
SC = BS // NCORES    # 128 rows per core
NBLK = SC // 16      # 8 blocks of 16 rows
NGRP = SC // 4       # 32 groups of 4 rows

F32 = mybir.dt.float32
BF16 = mybir.dt.bfloat16
F8E3 = mybir.dt.float8e3
NPBF16 = np.dtype(ml_dtypes.bfloat16)
NPF8E3 = np.dtype(ml_dtypes.float8_e3m4)

AX = mybir.AxisListType
ALU = mybir.AluOpType
ACTF = mybir.ActivationFunctionType

# f-chunk ranges for the G path / final projection.  c2 overlaps c1 so all
# three stationaries are full 128-col (FWL); wfx rows for the overlap are
# zeroed on host.
CH = [(0, 128), (128, 256), (160, 288)]

# txb free-dim layout: [0:4608) natural (16j x 288f), [4608:6656) t0
# (4g x 512jt), [6656:8704) t1.
OFF_T0 = 16 * DX     # 4608
OFF_T1 = OFF_T0 + 2048


def build(sc=SC):
    assert sc % 16 == 0
    nblk = sc // 16
    nc = bacc.Bacc()

    txt_d = nc.dram_tensor("txt", [nblk, 128, 4096], BF16, kind="ExternalInput")
    txn_d = nc.dram_tensor("txn", [nblk, 128, 4608], F8E3, kind="ExternalInput")
    tx2_d = nc.dram_tensor("tx2", [nblk, 36, 2048], BF16, kind="ExternalInput")
    qk01_d = nc.dram_tensor("qk01", [128, nblk, 256], BF16, kind="ExternalInput")
    qk2_d = nc.dram_tensor("qk2", [36, sc * H], BF16, kind="ExternalInput")
    wfx_d = nc.dram_tensor("wfx", [128, 24, DOUT], BF16, kind="ExternalInput")
    obias_d = nc.dram_tensor("obias", [sc, DOUT], F32, kind="ExternalInput")
    rmask_d = nc.dram_tensor("rmask", [sc, 1], F32, kind="ExternalInput")
    out_d = nc.dram_tensor("out", [sc, DOUT], F32, kind="ExternalOutput")

    with TileContext(nc) as tc:
        with (
            tc.tile_pool(name="const", bufs=1) as cp,
            tc.tile_pool(name="txtp", bufs=4) as txtp,
            tc.tile_pool(name="txnp", bufs=4) as txnp,
            tc.tile_pool(name="tx2p", bufs=4) as tx2p,
            tc.tile_pool(name="qkbp", bufs=4) as qkbp,
            tc.tile_pool(name="attnp", bufs=2) as attnp,
            tc.tile_pool(name="smallp", bufs=2) as smallp,
            tc.tile_pool(name="ps_l", bufs=2, space="PSUM") as ps_l,
            tc.tile_pool(name="ps_at", bufs=2, space="PSUM") as ps_at,
            tc.tile_pool(name="ps_g", bufs=2, space="PSUM") as ps_g,
            tc.tile_pool(name="ps_o", bufs=1, space="PSUM") as ps_o,
        ):
            # ---------------- constants ----------------
            qk2 = cp.tile([36, sc * H], BF16, name="qk2")
            nc.gpsimd.dma_start(out=qk2, in_=qk2_d[:, :])
            eye = cp.tile([128, 128], F32, name="eye")
            make_identity(nc, eye)
            obias = cp.tile([sc, DOUT], F32, name="obias")
            rmask = cp.tile([sc, 1], F32, name="rmask")
            gall = cp.tile([128, 24, sc], BF16, name="gall")

            # ---------------- main loop ----------------
            for blk in range(nblk):
                qkb = qkbp.tile([128, 256], BF16, tag="qkb", name="qkb")
                nc.sync.dma_start(out=qkb, in_=qk01_d[:, blk, :])
                txt = txtp.tile([128, 4096], BF16, tag="txt", name="txt")
                nsp = 4 if blk == 0 else 2
                for sp in range(nsp):
                    w = 4096 // nsp
                    nc.sync.dma_start(out=txt[:, sp * w:(sp + 1) * w],
                                      in_=txt_d[blk, :, sp * w:(sp + 1) * w])
                txn = txnp.tile([128, 4608], F8E3, tag="txn", name="txn")
                nc.scalar.dma_start(out=txn, in_=txn_d[blk, :, :])
                t2b = tx2p.tile([36, 2048], BF16, tag="t2b", name="t2b")
                nc.gpsimd.dma_start(out=t2b, in_=tx2_d[blk, :, :])

                l_ps = ps_l.tile([128, 512], F32, name="l_ps")
                for g4 in range(4):
                    g = blk * 4 + g4
                    csl = slice(g4 * 32, (g4 + 1) * 32)
                    c1sl = slice(128 + g4 * 32, 128 + (g4 + 1) * 32)
                    osl = slice(g4 * 32, (g4 + 1) * 32)
                    nc.tensor.matmul(
                        l_ps[osl, :], qkb[:, csl],
                        txt[:, g4 * 512:(g4 + 1) * 512],
                        start=True, stop=False, tile_position=(0, g4 * 32))
                    nc.tensor.matmul(
                        l_ps[osl, :], qkb[:, c1sl],
                        txt[:, 2048 + g4 * 512:2048 + (g4 + 1) * 512],
                        start=False, stop=False, tile_position=(0, g4 * 32))
                    q2sl = slice(g * 4 * H, (g + 1) * 4 * H)
                    nc.tensor.matmul(
                        l_ps[osl, :], qk2[:, q2sl],
                        t2b[:, g4 * 512:(g4 + 1) * 512],
                        start=False, stop=True, tile_position=(0, g4 * 32))

                # softmax over the 512-wide rows (off-window slots at -1e30)
                nmx = smallp.tile([128, 1], F32, tag="nmx", name="nmx")
                nc.vector.tensor_reduce(nmx, l_ps, axis=AX.X, op=ALU.max,
                                        negate=True)
                den = smallp.tile([128, 1], F32, tag="den", name="den")
                attn_e = attnp.tile([128, 512], F32, tag="ae", name="attn_e")
                nc.scalar.activation(attn_e, l_ps, ACTF.Exp, bias=nmx,
                                     scale=1.0, accum_out=den)
                rden = smallp.tile([128, 1], F32, tag="rden", name="rden")
                nc.vector.reciprocal(rden, den)
                attn_n = attnp.tile([128, 512], F32, tag="an", name="attn_n")
                nc.vector.tensor_scalar_mul(attn_n, attn_e, rden)

                # attn^T [t, (j,h)]: sum of 4 window transposes is exact
                # because off-window attn entries are exactly 0.
                at_ps = ps_at.tile([128, 128], F32, name="at_ps")
                for w in range(4):
                    nc.tensor.matmul(
                        at_ps, attn_n[:, w * 128:(w + 1) * 128], eye,
                        start=(w == 0), stop=(w == 3), is_transpose=True)
                atT = attnp.tile([128, 128], BF16, tag="atT", name="atT")
                nc.vector.tensor_copy(atT, at_ps)

                # G path: natural tgtx stationary (128-col, FWL), attT moving
                gt_ps = ps_g.tile([128, 24, 16], F32, name="gt_ps")
                for j in range(16):
                    av = atT[:, j * 8:(j + 1) * 8]
                    for c, (f0, f1) in enumerate(CH):
                        nc.tensor.matmul(
                            gt_ps[:, c * 8:(c + 1) * 8, j],
                            txn[:, j * DX + f0:j * DX + f1], av,
                            start=True, stop=True)
                bsl = slice(blk * 16, (blk + 1) * 16)
                nc.vector.tensor_copy(gall[:, :, bsl], gt_ps)

            # ---------------- output projection ----------------
            wfxt = cp.tile([128, 24, DOUT], BF16, name="wfxt")
            nc.gpsimd.dma_start(out=wfxt, in_=wfx_d[:, :, :])
            nc.gpsimd.dma_start(out=obias, in_=obias_d[:, :])
            nc.gpsimd.dma_start(out=rmask, in_=rmask_d[:, :])
            out_ps = ps_o.tile([sc, DOUT], F32, name="out_ps")
            for k in range(24):
                nc.tensor.matmul(
                    out_ps, gall[:, k, :], wfxt[:, k, :],
                    start=(k == 0), stop=(k == 23))
            out_sb = cp.tile([sc, DOUT], F32, name="out_sb")
            nc.vector.tensor_tensor(out_sb, out_ps, obias, op=ALU.add)
            out_sb2 = cp.tile([sc, DOUT], F32, name="out_sb2")
            nc.vector.tensor_scalar_mul(out_sb2, out_sb, rmask)
            nc.sync.dma_start(out=out_d[:, :], in_=out_sb2)

    nc.finalize()
    return nc


def host_prep(src, tgt, rpe, tgt_padding_mask, in_proj_weight, in_proj_bias,
              out_proj_weight, out_proj_bias, rpe_weight, rpe_bias):
    """Host-side folding + layout prep.  Returns per-core input maps."""
    f = np.float32
    scale = f(1.0 / np.sqrt(DH))

    src_f = np.asarray(src, f).reshape(BS, D)
    ipw = np.asarray(in_proj_weight, f)
    ipb = np.asarray(in_proj_bias, f)
    opw = np.asarray(out_proj_weight, f)
    opb = np.asarray(out_proj_bias, f)
    rw = np.asarray(rpe_weight, f)
    rb = np.asarray(rpe_bias, f)

    # ---- q-path fold (host): qk[(f|rpe|sel), s, h] ----
    q_s = (src_f @ ipw[:D].T + ipb[:D]) * scale          # [BS, D]
    wk = ipw[D:2 * D]                                    # [e, d]
    rwk = rw[:D]                                         # [e, r]
    qh = q_s.reshape(BS, H, DH)
    qw = np.einsum('shk,hkf->shf', qh, wk.reshape(H, DH, D))     # [BS,H,D]
    qrw = np.einsum('shk,hkf->shf', qh, rwk.reshape(H, DH, DR))  # [BS,H,DR]
    sel = (np.arange(4)[:, None] == (np.arange(SC) % 4)[None, :]).astype(f)
    qwT = qw.transpose(2, 0, 1).reshape(D, NCORES, NBLK, 128)  # [D,c,b,jh]
    qrwT = qrw.transpose(2, 0, 1).reshape(DR, NCORES, SC * H)
    qk01 = np.empty((NCORES, 128, NBLK, 256), NPBF16)
    qk01[:, :, :, 0:128] = qwT[0:128].transpose(1, 0, 2, 3).astype(NPBF16)
    qk01[:, :, :, 128:256] = qwT[128:256].transpose(1, 0, 2, 3).astype(NPBF16)
    qk2 = np.empty((NCORES, 36, SC * H), NPBF16)
    qk2[:, 0:32] = qrwT.transpose(1, 0, 2).astype(NPBF16)
    selh = np.broadcast_to(sel[:, :, None], (4, SC, H)).reshape(4, SC * H)
    qk2[:, 32:36] = selh.astype(NPBF16)[None]

    # ---- tgtx in both layouts (bf16) ----
    tgtx = np.concatenate(
        [np.asarray(tgt, f).reshape(BS, T, D),
         np.asarray(rpe, f).reshape(BS, T, DR)], axis=-1)   # [BS, T, DX]
    tgtx16 = tgtx.astype(NPBF16)
    # natural: txn[.., t, j*288+f] (fp8 e3m4 for the G path)
    txn = np.ascontiguousarray(tgtx.reshape(
        NCORES, NBLK, 16, T, DX).transpose(0, 1, 3, 2, 4).reshape(
        NCORES, NBLK, T, 16 * DX).astype(NPF8E3))
    # transposed: [c, blk, f, (g4, j, t)]
    txtT = tgtx16.reshape(NCORES, NBLK, 4, 4, T, DX).transpose(
        0, 1, 5, 2, 3, 4).reshape(NCORES, NBLK, DX, 2048)
    txt = np.empty((NCORES, NBLK, 128, 4096), NPBF16)
    txt[:, :, :, 0:2048] = txtT[:, :, 0:128]
    txt[:, :, :, 2048:4096] = txtT[:, :, 128:256]
    tx2 = np.empty((NCORES, NBLK, 36, 2048), NPBF16)
    tx2[:, :, 0:32] = txtT[:, :, 256:288]

    # ---- mask rows: M[m, (g4, j, t)] = maskadd if j==m else -1e30 ----
    mask = np.asarray(tgt_padding_mask, bool).reshape(BS, T)
    no_valid = mask.all(-1)
    maskadd = np.where(mask & ~no_valid[:, None], f(-1e30), f(0.0))
    Mfull = np.full((BS, 4, T), -1e30, f).reshape(NCORES, NBLK, 4, 4, 4, T)
    ma_g = maskadd.reshape(NCORES, NBLK, 4, 4, T)
    for m in range(4):
        Mfull[:, :, :, m, m, :] = ma_g[:, :, :, m, :]
    # Mfull dims: [c, blk, g4, m, j, t] -> [c, blk, m, (g4, j, t)]
    tx2[:, :, 32:36] = Mfull.transpose(0, 1, 3, 2, 4, 5).reshape(
        NCORES, NBLK, 4, 2048).astype(NPBF16)

    # ---- output-side folds ----
    wvx = np.concatenate([ipw[2 * D:3 * D], rw[D:2 * D]], axis=1)  # [e, DX]
    wfxh = np.empty((H, DX, DOUT), f)
    for h in range(H):
        hs = slice(h * DH, (h + 1) * DH)
        wfxh[h] = (opw[:, hs] @ wvx[hs, :]).T
    wfxk = np.zeros((24, 128, DOUT), f)
    for h in range(H):
        wfxk[h] = wfxh[h, CH[0][0]:CH[0][1]]
        wfxk[8 + h] = wfxh[h, CH[1][0]:CH[1][1]]
        wfxk[16 + h, 96:128] = wfxh[h, D:DX]     # rows 160:256 stay zero
    wfxk16 = np.ascontiguousarray(
        wfxk.transpose(1, 0, 2).astype(NPBF16))  # [128, 24, 256]

    obias = (opb + opw @ (ipb[2 * D:3 * D] + rb[D:2 * D]))[None, :]
    obias = np.ascontiguousarray(np.repeat(obias.astype(f), SC, axis=0))
    rowmask = np.ascontiguousarray((~no_valid).astype(f)[:, None])

    in_maps = []
    for c in range(NCORES):
        sl = slice(c * SC, (c + 1) * SC)
        in_maps.append({
            "txt": np.ascontiguousarray(txt[c]),
            "txn": np.ascontiguousarray(txn[c]),
            "tx2": np.ascontiguousarray(tx2[c]),
            "qk01": np.ascontiguousarray(qk01[c]),
            "qk2": np.ascontiguousarray(qk2[c]),
            "wfx": wfxk16,
            "obias": obias,
            "rmask": rowmask[sl],
        })
    return in_maps


_NC_CACHE = {}


def get_nc(sc=SC):
    if sc not in _NC_CACHE:
        _NC_CACHE[sc] = build(sc)
    return _NC_CACHE[sc]


def run(in_maps, trace=False):
    nc = get_nc(SC)
    return run_bass_kernel_spmd(nc, in_maps, list(range(NCORES)), trace=trace)


def kernel(**inputs):
    in_maps = host_prep(**inputs)
    res = run(in_maps).results
    out = np.concatenate([res[c]["out"] for c in range(NCORES)], axis=0)
    return np.ascontiguousarray(out.reshape(B, S, D))


# revision 19
# speedup vs baseline: 1.0543x; 1.0543x over previous
"""AttentionRPE kernel for 8 Trainium2 NeuronCores — bf16 single-pass design.

Math (per (b,s) row, T=128 targets, D=256, H=8 heads, DH=32, DR=32):
  q   = src @ Wsrc.T + bsrc                       [D]
  K'  = tgt @ Wk.T + rpe @ Rwk.T                  [T, D]
  att = softmax_h(q_h . K'_h / sqrt(DH))          [H, T]   (masked)
  out = (att @ V')_heads @ Wout.T + bout          [D]

Device formulation:
  * The whole q-path is linear in src, so qw = (q/sqrt(DH)) @ Wkx is folded
    ON HOST into per-row stationary vectors qk[(s,h), f] (f = 288 tgt|rpe
    features).  logits[(s,h), t] = sum_f qk[f,(s,h)] * tgtxT[f, t].
  * Host ships tgtx in BOTH layouts as bf16: transposed (for the logits
    moving operand) and natural (stationary for the G path).  No on-chip
    transposes of the big tensor; all matmuls are single-pass bf16 (FWL
    fast-weight-load kicks in on the 128-col stationaries).
  * Padding mask + off-window garbage masking folded into the 3rd logits
    matmul as 4 extra stationary rows (per-group one-hot selector).
  * G[f, j, h] = sum_t tgtx[t,f] * att[h,t] via natural-tgtx stationary +
    transposed-attention moving (8 cols/row).  Final: out = sum_k
    gall[:,k,:].T @ wfx[k] with host-folded wfx = (Wout_h @ Wvx_h).T.
  * All per-block data rides in 2 large DMAs (one per HWDGE engine) —
    HWDGE dispatch is ~0.7us/call on the issuing engine queue, so DMA
    count is minimized.

Sharding: 1024 (b,s) rows split contiguously over 8 cores (128 each).
"""

import numpy as np
import ml_dtypes

import concourse.bass as bass
import concourse.bacc as bacc
import concourse.mybir as mybir
from concourse.tile import TileContext
from concourse.masks import make_identity
from concourse.bass_utils import run_bass_kernel_spmd

B, S, T, D = 2, 512, 128, 256
H, DH, DR = 8, 32, 32
DX = D + DR          # 288 = tgt|rpe feature dim
DOUT = D
NCORES = 8
BS = B * S           # 1024 total rows
SC = BS // NCORES    # 128 rows per core
NBLK = SC // 16      # 8 blocks of 16 rows
NGRP = SC // 4       # 32 groups of 4 rows

F32 = mybir.dt.float32
BF16 = mybir.dt.bfloat16
F8E3 = mybir.dt.float8e3
NPBF16 = np.dtype(ml_dtypes.bfloat16)
NPF8E3 = np.dtype(ml_dtypes.float8_e3m4)

AX = mybir.AxisListType
ALU = mybir.AluOpType
ACTF = mybir.ActivationFunctionType

# f-chunk ranges for the G path / final projection.  c2 overlaps c1 so all
# three stationaries are full 128-col (FWL); wfx rows for the overlap are
# zeroed on host.
CH = [(0, 128), (128, 256), (160, 288)]

# txb free-dim layout: [0:4608) natural (16j x 288f), [4608:6656) t0
# (4g x 512jt), [6656:8704) t1.
OFF_T0 = 16 * DX     # 4608
OFF_T1 = OFF_T0 + 2048


def build(sc=SC):
    assert sc % 16 == 0
    nblk = sc // 16
    nc = bacc.Bacc()

    txt_d = nc.dram_tensor("txt", [nblk, 128, 4096], BF16, kind="ExternalInput")
    txn_d = nc.dram_tensor("txn", [nblk, 128, 4608], F8E3, kind="ExternalInput")
    tx2_d = nc.dram_tensor("tx2", [nblk, 36, 2048], BF16, kind="ExternalInput")
    qk01_d = nc.dram_tensor("qk01", [128, 2 * sc * H], BF16, kind="ExternalInput")
    qk2_d = nc.dram_tensor("qk2", [36, sc * H], BF16, kind="ExternalInput")
    wfx_d = nc.dram_tensor("wfx", [128, 24, DOUT], BF16, kind="ExternalInput")
    obias_d = nc.dram_tensor("obias", [sc, DOUT], F32, kind="ExternalInput")
    rmask_d = nc.dram_tensor("rmask", [sc, 1], F32, kind="ExternalInput")
    out_d = nc.dram_tensor("out", [sc, DOUT], F32, kind="ExternalOutput")

    with TileContext(nc) as tc:
        with (
            tc.tile_pool(name="const", bufs=1) as cp,
            tc.tile_pool(name="txtp", bufs=4) as txtp,
            tc.tile_pool(name="txnp", bufs=4) as txnp,
            tc.tile_pool(name="tx2p", bufs=4) as tx2p,
            tc.tile_pool(name="attnp", bufs=2) as attnp,
            tc.tile_pool(name="smallp", bufs=2) as smallp,
            tc.tile_pool(name="ps_l", bufs=2, space="PSUM") as ps_l,
            tc.tile_pool(name="ps_at", bufs=2, space="PSUM") as ps_at,
            tc.tile_pool(name="ps_g", bufs=2, space="PSUM") as ps_g,
            tc.tile_pool(name="ps_o", bufs=1, space="PSUM") as ps_o,
        ):
            # ---------------- constants ----------------
            qk01 = cp.tile([128, 2 * sc * H], BF16, name="qk01")
            nc.gpsimd.dma_start(out=qk01, in_=qk01_d[:, :])
            qk2 = cp.tile([36, sc * H], BF16, name="qk2")
            nc.gpsimd.dma_start(out=qk2, in_=qk2_d[:, :])
            eye = cp.tile([128, 128], F32, name="eye")
            make_identity(nc, eye)
            obias = cp.tile([sc, DOUT], F32, name="obias")
            nc.gpsimd.dma_start(out=obias, in_=obias_d[:, :])
            rmask = cp.tile([sc, 1], F32, name="rmask")
            nc.gpsimd.dma_start(out=rmask, in_=rmask_d[:, :])
            gall = cp.tile([128, 24, sc], BF16, name="gall")

            # ---------------- main loop ----------------
            for blk in range(nblk):
                txt = txtp.tile([128, 4096], BF16, tag="txt", name="txt")
                nc.sync.dma_start(out=txt[:, 0:2048], in_=txt_d[blk, :, 0:2048])
                nc.sync.dma_start(out=txt[:, 2048:4096],
                                  in_=txt_d[blk, :, 2048:4096])
                txn = txnp.tile([128, 4608], F8E3, tag="txn", name="txn")
                nc.scalar.dma_start(out=txn, in_=txn_d[blk, :, :])
                t2b = tx2p.tile([36, 2048], BF16, tag="t2b", name="t2b")
                nc.gpsimd.dma_start(out=t2b, in_=tx2_d[blk, :, :])

                l_ps = ps_l.tile([128, 512], F32, name="l_ps")
                for g4 in range(4):
                    g = blk * 4 + g4
                    csl = slice(g * 4 * H, (g + 1) * 4 * H)
                    c1sl = slice(sc * H + g * 4 * H, sc * H + (g + 1) * 4 * H)
                    osl = slice(g4 * 32, (g4 + 1) * 32)
                    nc.tensor.matmul(
                        l_ps[osl, :], qk01[:, csl],
                        txt[:, g4 * 512:(g4 + 1) * 512],
                        start=True, stop=False, tile_position=(0, g4 * 32))
                    nc.tensor.matmul(
                        l_ps[osl, :], qk01[:, c1sl],
                        txt[:, 2048 + g4 * 512:2048 + (g4 + 1) * 512],
                        start=False, stop=False, tile_position=(0, g4 * 32))
                    q2sl = slice(g * 4 * H, (g + 1) * 4 * H)
                    nc.tensor.matmul(
                        l_ps[osl, :], qk2[:, q2sl],
                        t2b[:, g4 * 512:(g4 + 1) * 512],
                        start=False, stop=True, tile_position=(0, g4 * 32))

                # softmax over the 512-wide rows (off-window slots at -1e30)
                nmx = smallp.tile([128, 1], F32, tag="nmx", name="nmx")
                nc.vector.tensor_reduce(nmx, l_ps, axis=AX.X, op=ALU.max,
                                        negate=True)
                den = smallp.tile([128, 1], F32, tag="den", name="den")
                attn_e = attnp.tile([128, 512], F32, tag="ae", name="attn_e")
                nc.scalar.activation(attn_e, l_ps, ACTF.Exp, bias=nmx,
                                     scale=1.0, accum_out=den)
                rden = smallp.tile([128, 1], F32, tag="rden", name="rden")
                nc.vector.reciprocal(rden, den)
                attn_n = attnp.tile([128, 512], F32, tag="an", name="attn_n")
                nc.vector.tensor_scalar_mul(attn_n, attn_e, rden)

                # attn^T [t, (j,h)]: sum of 4 window transposes is exact
                # because off-window attn entries are exactly 0.
                at_ps = ps_at.tile([128, 128], F32, name="at_ps")
                for w in range(4):
                    nc.tensor.matmul(
                        at_ps, attn_n[:, w * 128:(w + 1) * 128], eye,
                        start=(w == 0), stop=(w == 3), is_transpose=True)
                atT = attnp.tile([128, 128], BF16, tag="atT", name="atT")
                nc.vector.tensor_copy(atT, at_ps)

                # G path: natural tgtx stationary (128-col, FWL), attT moving
                gt_ps = ps_g.tile([128, 24, 16], F32, name="gt_ps")
                for j in range(16):
                    av = atT[:, j * 8:(j + 1) * 8]
                    for c, (f0, f1) in enumerate(CH):
                        nc.tensor.matmul(
                            gt_ps[:, c * 8:(c + 1) * 8, j],
                            txn[:, j * DX + f0:j * DX + f1], av,
                            start=True, stop=True)
                bsl = slice(blk * 16, (blk + 1) * 16)
                nc.vector.tensor_copy(gall[:, :, bsl], gt_ps)

            # ---------------- output projection ----------------
            wfxt = cp.tile([128, 24, DOUT], BF16, name="wfxt")
            nc.gpsimd.dma_start(out=wfxt, in_=wfx_d[:, :, :])
            out_ps = ps_o.tile([sc, DOUT], F32, name="out_ps")
            for k in range(24):
                nc.tensor.matmul(
                    out_ps, gall[:, k, :], wfxt[:, k, :],
                    start=(k == 0), stop=(k == 23))
            out_sb = cp.tile([sc, DOUT], F32, name="out_sb")
            nc.vector.tensor_tensor(out_sb, out_ps, obias, op=ALU.add)
            out_sb2 = cp.tile([sc, DOUT], F32, name="out_sb2")
            nc.vector.tensor_scalar_mul(out_sb2, out_sb, rmask)
            nc.sync.dma_start(out=out_d[:, :], in_=out_sb2)

    nc.finalize()
    return nc


def host_prep(src, tgt, rpe, tgt_padding_mask, in_proj_weight, in_proj_bias,
              out_proj_weight, out_proj_bias, rpe_weight, rpe_bias):
    """Host-side folding + layout prep.  Returns per-core input maps."""
    f = np.float32
    scale = f(1.0 / np.sqrt(DH))

    src_f = np.asarray(src, f).reshape(BS, D)
    ipw = np.asarray(in_proj_weight, f)
    ipb = np.asarray(in_proj_bias, f)
    opw = np.asarray(out_proj_weight, f)
    opb = np.asarray(out_proj_bias, f)
    rw = np.asarray(rpe_weight, f)
    rb = np.asarray(rpe_bias, f)

    # ---- q-path fold (host): qk[(f|rpe|sel), s, h] ----
    q_s = (src_f @ ipw[:D].T + ipb[:D]) * scale          # [BS, D]
    wk = ipw[D:2 * D]                                    # [e, d]
    rwk = rw[:D]                                         # [e, r]
    qh = q_s.reshape(BS, H, DH)
    qw = np.einsum('shk,hkf->shf', qh, wk.reshape(H, DH, D))     # [BS,H,D]
    qrw = np.einsum('shk,hkf->shf', qh, rwk.reshape(H, DH, DR))  # [BS,H,DR]
    sel = (np.arange(4)[:, None] == (np.arange(SC) % 4)[None, :]).astype(f)
    qwT = qw.transpose(2, 0, 1).reshape(D, NCORES, SC * H)    # [D, c, s*h]
    qrwT = qrw.transpose(2, 0, 1).reshape(DR, NCORES, SC * H)
    qk01 = np.empty((NCORES, 128, 2 * SC * H), NPBF16)
    qk01[:, :, 0:SC * H] = qwT[0:128].transpose(1, 0, 2).astype(NPBF16)
    qk01[:, :, SC * H:] = qwT[128:256].transpose(1, 0, 2).astype(NPBF16)
    qk2 = np.empty((NCORES, 36, SC * H), NPBF16)
    qk2[:, 0:32] = qrwT.transpose(1, 0, 2).astype(NPBF16)
    selh = np.broadcast_to(sel[:, :, None], (4, SC, H)).reshape(4, SC * H)
    qk2[:, 32:36] = selh.astype(NPBF16)[None]

    # ---- tgtx in both layouts (bf16) ----
    tgtx = np.concatenate(
        [np.asarray(tgt, f).reshape(BS, T, D),
         np.asarray(rpe, f).reshape(BS, T, DR)], axis=-1)   # [BS, T, DX]
    tgtx16 = tgtx.astype(NPBF16)
    # natural: txn[.., t, j*288+f] (fp8 e3m4 for the G path)
    txn = np.ascontiguousarray(tgtx.reshape(
        NCORES, NBLK, 16, T, DX).transpose(0, 1, 3, 2, 4).reshape(
        NCORES, NBLK, T, 16 * DX).astype(NPF8E3))
    # transposed: [c, blk, f, (g4, j, t)]
    txtT = tgtx16.reshape(NCORES, NBLK, 4, 4, T, DX).transpose(
        0, 1, 5, 2, 3, 4).reshape(NCORES, NBLK, DX, 2048)
    txt = np.empty((NCORES, NBLK, 128, 4096), NPBF16)
    txt[:, :, :, 0:2048] = txtT[:, :, 0:128]
    txt[:, :, :, 2048:4096] = txtT[:, :, 128:256]
    tx2 = np.empty((NCORES, NBLK, 36, 2048), NPBF16)
    tx2[:, :, 0:32] = txtT[:, :, 256:288]

    # ---- mask rows: M[m, (g4, j, t)] = maskadd if j==m else -1e30 ----
    mask = np.asarray(tgt_padding_mask, bool).reshape(BS, T)
    no_valid = mask.all(-1)
    maskadd = np.where(mask & ~no_valid[:, None], f(-1e30), f(0.0))
    Mfull = np.full((BS, 4, T), -1e30, f).reshape(NCORES, NBLK, 4, 4, 4, T)
    ma_g = maskadd.reshape(NCORES, NBLK, 4, 4, T)
    for m in range(4):
        Mfull[:, :, :, m, m, :] = ma_g[:, :, :, m, :]
    # Mfull dims: [c, blk, g4, m, j, t] -> [c, blk, m, (g4, j, t)]
    tx2[:, :, 32:36] = Mfull.transpose(0, 1, 3, 2, 4, 5).reshape(
        NCORES, NBLK, 4, 2048).astype(NPBF16)

    # ---- output-side folds ----
    wvx = np.concatenate([ipw[2 * D:3 * D], rw[D:2 * D]], axis=1)  # [e, DX]
    wfxh = np.empty((H, DX, DOUT), f)
    for h in range(H):
        hs = slice(h * DH, (h + 1) * DH)
        wfxh[h] = (opw[:, hs] @ wvx[hs, :]).T
    wfxk = np.zeros((24, 128, DOUT), f)
    for h in range(H):
        wfxk[h] = wfxh[h, CH[0][0]:CH[0][1]]
        wfxk[8 + h] = wfxh[h, CH[1][0]:CH[1][1]]
        wfxk[16 + h, 96:128] = wfxh[h, D:DX]     # rows 160:256 stay zero
    wfxk16 = np.ascontiguousarray(
        wfxk.transpose(1, 0, 2).astype(NPBF16))  # [128, 24, 256]

    obias = (opb + opw @ (ipb[2 * D:3 * D] + rb[D:2 * D]))[None, :]
    obias = np.ascontiguousarray(np.repeat(obias.astype(f), SC, axis=0))
    rowmask = np.ascontiguousarray((~no_valid).astype(f)[:, None])

    in_maps = []
    for c in range(NCORES):
        sl = slice(c * SC, (c + 1) * SC)
        in_maps.append({
            "txt": np.ascontiguousarray(txt[c]),
            "txn": np.ascontiguousarray(txn[c]),
            "tx2": np.ascontiguousarray(tx2[c]),
            "qk01": np.ascontiguousarray(qk01[c]),
            "qk2": np.ascontiguousarray(qk2[c]),
            "wfx": wfxk16,
            "obias": obias,
            "rmask": rowmask[sl],
        })
    return in_maps


_NC_CACHE = {}


def get_nc(sc=SC):
    if sc not in _NC_CACHE:
        _NC_CACHE[sc] = build(sc)
    return _NC_CACHE[sc]


def run(in_maps, trace=False):
    nc = get_nc(SC)
    return run_bass_kernel_spmd(nc, in_maps, list(range(NCORES)), trace=trace)


def kernel(**inputs):
    in_maps = host_prep(**inputs)
    res = run(in_maps).results
    out = np.concatenate([res[c]["out"] for c in range(NCORES)], axis=0)
    return np.ascontiguousarray(out.reshape(B, S, D))


# revision 21
# speedup vs baseline: 1.2013x; 1.1394x over previous
"""AttentionRPE kernel for 8 Trainium2 NeuronCores — bf16 single-pass design.

Math (per (b,s) row, T=128 targets, D=256, H=8 heads, DH=32, DR=32):
  q   = src @ Wsrc.T + bsrc                       [D]
  K'  = tgt @ Wk.T + rpe @ Rwk.T                  [T, D]
  att = softmax_h(q_h . K'_h / sqrt(DH))          [H, T]   (masked)
  out = (att @ V')_heads @ Wout.T + bout          [D]

Device formulation:
  * The whole q-path is linear in src, so qw = (q/sqrt(DH)) @ Wkx is folded
    ON HOST into per-row stationary vectors qk[(s,h), f] (f = 288 tgt|rpe
    features).  logits[(s,h), t] = sum_f qk[f,(s,h)] * tgtxT[f, t].
  * Host ships tgtx in BOTH layouts as bf16: transposed (for the logits
    moving operand) and natural (stationary for the G path).  No on-chip
    transposes of the big tensor; all matmuls are single-pass bf16 (FWL
    fast-weight-load kicks in on the 128-col stationaries).
  * Padding mask + off-window garbage masking folded into the 3rd logits
    matmul as 4 extra stationary rows (per-group one-hot selector).
  * G[f, j, h] = sum_t tgtx[t,f] * att[h,t] via natural-tgtx stationary +
    transposed-attention moving (8 cols/row).  Final: out = sum_k
    gall[:,k,:].T @ wfx[k] with host-folded wfx = (Wout_h @ Wvx_h).T.
  * All per-block data rides in 2 large DMAs (one per HWDGE engine) —
    HWDGE dispatch is ~0.7us/call on the issuing engine queue, so DMA
    count is minimized.

Sharding: 1024 (b,s) rows split contiguously over 8 cores (128 each).
"""

import numpy as np
import ml_dtypes

import concourse.bass as bass
import concourse.bacc as bacc
import concourse.mybir as mybir
from concourse.tile import TileContext
from concourse.masks import make_identity
from concourse.bass_utils import run_bass_kernel_spmd

B, S, T, D = 2, 512, 128, 256
H, DH, DR = 8, 32, 32
DX = D + DR          # 288 = tgt|rpe feature dim
DOUT = D
NCORES = 8
BS = B * S           # 1024 total rows
SC = BS // NCORES    # 128 rows per core
NBLK = SC // 16      # 8 blocks of 16 rows
NGRP = SC // 4       # 32 groups of 4 rows

F32 = mybir.dt.float32
BF16 = mybir.dt.bfloat16
F8E3 = mybir.dt.float8e3
NPBF16 = np.dtype(ml_dtypes.bfloat16)
NPF8E3 = np.dtype(ml_dtypes.float8_e3m4)

AX = mybir.AxisListType
ALU = mybir.AluOpType
ACTF = mybir.ActivationFunctionType

# f-chunk ranges for the G path / final projection.  c2 overlaps c1 so all
# three stationaries are full 128-col (FWL); wfx rows for the overlap are
# zeroed on host.
CH = [(0, 128), (128, 256), (160, 288)]

# txb free-dim layout: [0:4608) natural (16j x 288f), [4608:6656) t0
# (4g x 512jt), [6656:8704) t1.
OFF_T0 = 16 * DX     # 4608
OFF_T1 = OFF_T0 + 2048


def build(sc=SC):
    assert sc % 16 == 0
    nblk = sc // 16
    nc = bacc.Bacc()

    txt_d = nc.dram_tensor("txt", [nblk, 128, 4096], BF16, kind="ExternalInput")
    txn_d = nc.dram_tensor("txn", [nblk, 128, 4608], F8E3, kind="ExternalInput")
    tx2_d = nc.dram_tensor("tx2", [nblk, 36, 2048], BF16, kind="ExternalInput")
    qk01_d = nc.dram_tensor("qk01", [128, 2 * sc * H], BF16, kind="ExternalInput")
    qk2_d = nc.dram_tensor("qk2", [36, sc * H], BF16, kind="ExternalInput")
    wfx_d = nc.dram_tensor("wfx", [128, 24, DOUT], BF16, kind="ExternalInput")
    obias_d = nc.dram_tensor("obias", [sc, DOUT], F32, kind="ExternalInput")
    rmask_d = nc.dram_tensor("rmask", [sc, 1], F32, kind="ExternalInput")
    out_d = nc.dram_tensor("out", [sc, DOUT], F32, kind="ExternalOutput")

    with TileContext(nc) as tc:
        with (
            tc.tile_pool(name="const", bufs=1) as cp,
            tc.tile_pool(name="txtp", bufs=4) as txtp,
            tc.tile_pool(name="txnp", bufs=4) as txnp,
            tc.tile_pool(name="tx2p", bufs=4) as tx2p,
            tc.tile_pool(name="attnp", bufs=2) as attnp,
            tc.tile_pool(name="smallp", bufs=2) as smallp,
            tc.tile_pool(name="ps_l", bufs=2, space="PSUM") as ps_l,
            tc.tile_pool(name="ps_at", bufs=2, space="PSUM") as ps_at,
            tc.tile_pool(name="ps_g", bufs=2, space="PSUM") as ps_g,
            tc.tile_pool(name="ps_o", bufs=1, space="PSUM") as ps_o,
        ):
            # ---------------- constants ----------------
            qk01 = cp.tile([128, 2 * sc * H], BF16, name="qk01")
            nc.gpsimd.dma_start(out=qk01, in_=qk01_d[:, :])
            qk2 = cp.tile([36, sc * H], BF16, name="qk2")
            nc.gpsimd.dma_start(out=qk2, in_=qk2_d[:, :])
            eye = cp.tile([128, 128], F32, name="eye")
            make_identity(nc, eye)
            obias = cp.tile([sc, DOUT], F32, name="obias")
            nc.gpsimd.dma_start(out=obias, in_=obias_d[:, :])
            rmask = cp.tile([sc, 1], F32, name="rmask")
            nc.gpsimd.dma_start(out=rmask, in_=rmask_d[:, :])
            gall = cp.tile([128, 24, sc], BF16, name="gall")

            # ---------------- main loop ----------------
            for blk in range(nblk):
                txt = txtp.tile([128, 4096], BF16, tag="txt", name="txt")
                nc.sync.dma_start(out=txt[:, 0:2048], in_=txt_d[blk, :, 0:2048])
                nc.sync.dma_start(out=txt[:, 2048:4096],
                                  in_=txt_d[blk, :, 2048:4096])
                txn = txnp.tile([128, 4608], F8E3, tag="txn", name="txn")
                nc.scalar.dma_start(out=txn, in_=txn_d[blk, :, :])
                t2b = tx2p.tile([36, 2048], BF16, tag="t2b", name="t2b")
                nc.gpsimd.dma_start(out=t2b, in_=tx2_d[blk, :, :])

                l_ps = ps_l.tile([128, 512], F32, name="l_ps")
                for g4 in range(4):
                    g = blk * 4 + g4
                    csl = slice(g * 4 * H, (g + 1) * 4 * H)
                    c1sl = slice(sc * H + g * 4 * H, sc * H + (g + 1) * 4 * H)
                    osl = slice(g4 * 32, (g4 + 1) * 32)
                    nc.tensor.matmul(
                        l_ps[osl, :], qk01[:, csl],
                        txt[:, g4 * 512:(g4 + 1) * 512],
                        start=True, stop=False, tile_position=(0, g4 * 32))
                    nc.tensor.matmul(
                        l_ps[osl, :], qk01[:, c1sl],
                        txt[:, 2048 + g4 * 512:2048 + (g4 + 1) * 512],
                        start=False, stop=False, tile_position=(0, g4 * 32))
                    q2sl = slice(g * 4 * H, (g + 1) * 4 * H)
                    nc.tensor.matmul(
                        l_ps[osl, :], qk2[:, q2sl],
                        t2b[:, g4 * 512:(g4 + 1) * 512],
                        start=False, stop=True, tile_position=(0, g4 * 32))

                # softmax over the 512-wide rows (off-window slots at -1e30)
                nmx = smallp.tile([128, 1], F32, tag="nmx", name="nmx")
                nc.vector.tensor_reduce(nmx, l_ps, axis=AX.X, op=ALU.max,
                                        negate=True)
                den = smallp.tile([128, 1], F32, tag="den", name="den")
                attn_e = attnp.tile([128, 512], F32, tag="ae", name="attn_e")
                nc.scalar.activation(attn_e, l_ps, ACTF.Exp, bias=nmx,
                                     scale=1.0, accum_out=den)
                rden = smallp.tile([128, 1], F32, tag="rden", name="rden")
                nc.vector.reciprocal(rden, den)
                attn_n = attnp.tile([128, 512], F32, tag="an", name="attn_n")
                nc.vector.tensor_scalar_mul(attn_n, attn_e, rden)

                # attn^T [t, (j,h)]: sum of 4 window transposes is exact
                # because off-window attn entries are exactly 0.
                at_ps = ps_at.tile([128, 128], F32, name="at_ps")
                for w in range(4):
                    nc.tensor.matmul(
                        at_ps, attn_n[:, w * 128:(w + 1) * 128], eye,
                        start=(w == 0), stop=(w == 3), is_transpose=True)
                atT = attnp.tile([128, 128], BF16, tag="atT", name="atT")
                nc.vector.tensor_copy(atT, at_ps)

                # G path: natural tgtx stationary (128-col, FWL), attT moving
                gt_ps = ps_g.tile([128, 24, 16], F32, name="gt_ps")
                for j in range(16):
                    av = atT[:, j * 8:(j + 1) * 8]
                    for c, (f0, f1) in enumerate(CH):
                        nc.tensor.matmul(
                            gt_ps[:, c * 8:(c + 1) * 8, j],
                            txn[:, j * DX + f0:j * DX + f1], av,
                            start=True, stop=True)
                bsl = slice(blk * 16, (blk + 1) * 16)
                nc.vector.tensor_copy(gall[:, :, bsl], gt_ps)

            # ---------------- output projection ----------------
            wfxt = cp.tile([128, 24, DOUT], BF16, name="wfxt")
            nc.gpsimd.dma_start(out=wfxt, in_=wfx_d[:, :, :])
            out_ps = ps_o.tile([sc, DOUT], F32, name="out_ps")
            for k in range(24):
                nc.tensor.matmul(
                    out_ps, gall[:, k, :], wfxt[:, k, :],
                    start=(k == 0), stop=(k == 23))
            out_sb = cp.tile([sc, DOUT], F32, name="out_sb")
            nc.vector.tensor_tensor(out_sb, out_ps, obias, op=ALU.add)
            out_sb2 = cp.tile([sc, DOUT], F32, name="out_sb2")
            nc.vector.tensor_scalar_mul(out_sb2, out_sb, rmask)
            nc.sync.dma_start(out=out_d[:, :], in_=out_sb2)

    nc.finalize()
    return nc


def host_prep(src, tgt, rpe, tgt_padding_mask, in_proj_weight, in_proj_bias,
              out_proj_weight, out_proj_bias, rpe_weight, rpe_bias):
    """Host-side folding + layout prep.  Returns per-core input maps."""
    f = np.float32
    scale = f(1.0 / np.sqrt(DH))

    src_f = np.asarray(src, f).reshape(BS, D)
    ipw = np.asarray(in_proj_weight, f)
    ipb = np.asarray(in_proj_bias, f)
    opw = np.asarray(out_proj_weight, f)
    opb = np.asarray(out_proj_bias, f)
    rw = np.asarray(rpe_weight, f)
    rb = np.asarray(rpe_bias, f)

    # ---- q-path fold (host): qk[(f|rpe|sel), s, h] ----
    q_s = (src_f @ ipw[:D].T + ipb[:D]) * scale          # [BS, D]
    wk = ipw[D:2 * D]                                    # [e, d]
    rwk = rw[:D]                                         # [e, r]
    qh = q_s.reshape(BS, H, DH)
    qw = np.einsum('shk,hkf->shf', qh, wk.reshape(H, DH, D))     # [BS,H,D]
    qrw = np.einsum('shk,hkf->shf', qh, rwk.reshape(H, DH, DR))  # [BS,H,DR]
    sel = (np.arange(4)[:, None] == (np.arange(SC) % 4)[None, :]).astype(f)
    qwT = qw.transpose(2, 0, 1).reshape(D, NCORES, SC * H)    # [D, c, s*h]
    qrwT = qrw.transpose(2, 0, 1).reshape(DR, NCORES, SC * H)
    qk01 = np.empty((NCORES, 128, 2 * SC * H), NPBF16)
    qk01[:, :, 0:SC * H] = qwT[0:128].transpose(1, 0, 2).astype(NPBF16)
    qk01[:, :, SC * H:] = qwT[128:256].transpose(1, 0, 2).astype(NPBF16)
    qk2 = np.empty((NCORES, 36, SC * H), NPBF16)
    qk2[:, 0:32] = qrwT.transpose(1, 0, 2).astype(NPBF16)
    selh = np.broadcast_to(sel[:, :, None], (4, SC, H)).reshape(4, SC * H)
    qk2[:, 32:36] = selh.astype(NPBF16)[None]

    # ---- tgtx in both layouts (bf16) ----
    tgtx = np.concatenate(
        [np.asarray(tgt, f).reshape(BS, T, D),
         np.asarray(rpe, f).reshape(BS, T, DR)], axis=-1)   # [BS, T, DX]
    tgtx16 = tgtx.astype(NPBF16)
    # natural: txn[.., t, j*288+f] (fp8 e3m4 for the G path)
    txn = np.ascontiguousarray(tgtx.reshape(
        NCORES, NBLK, 16, T, DX).transpose(0, 1, 3, 2, 4).reshape(
        NCORES, NBLK, T, 16 * DX).astype(NPF8E3))
    # transposed: [c, blk, f, (g4, j, t)]
    txtT = tgtx16.reshape(NCORES, NBLK, 4, 4, T, DX).transpose(
        0, 1, 5, 2, 3, 4).reshape(NCORES, NBLK, DX, 2048)
    txt = np.empty((NCORES, NBLK, 128, 4096), NPBF16)
    txt[:, :, :, 0:2048] = txtT[:, :, 0:128]
    txt[:, :, :, 2048:4096] = txtT[:, :, 128:256]
    tx2 = np.empty((NCORES, NBLK, 36, 2048), NPBF16)
    tx2[:, :, 0:32] = txtT[:, :, 256:288]

    # ---- mask rows: M[m, (g4, j, t)] = maskadd if j==m else -1e30 ----
    mask = np.asarray(tgt_padding_mask, bool).reshape(BS, T)
    no_valid = mask.all(-1)
    maskadd = np.where(mask & ~no_valid[:, None], f(-1e30), f(0.0))
    Mfull = np.full((BS, 4, T), -1e30, f).reshape(NCORES, NBLK, 4, 4, 4, T)
    ma_g = maskadd.reshape(NCORES, NBLK, 4, 4, T)
    for m in range(4):
        Mfull[:, :, :, m, m, :] = ma_g[:, :, :, m, :]
    # Mfull dims: [c, blk, g4, m, j, t] -> [c, blk, m, (g4, j, t)]
    tx2[:, :, 32:36] = Mfull.transpose(0, 1, 3, 2, 4, 5).reshape(
        NCORES, NBLK, 4, 2048).astype(NPBF16)

    # ---- output-side folds ----
    wvx = np.concatenate([ipw[2 * D:3 * D], rw[D:2 * D]], axis=1)  # [e, DX]
    wfxh = np.empty((H, DX, DOUT), f)
    for h in range(H):
        hs = slice(h * DH, (h + 1) * DH)
        wfxh[h] = (opw[:, hs] @ wvx[hs, :]).T
    wfxk = np.zeros((24, 128, DOUT), f)
    for h in range(H):
        wfxk[h] = wfxh[h, CH[0][0]:CH[0][1]]
        wfxk[8 + h] = wfxh[h, CH[1][0]:CH[1][1]]
        wfxk[16 + h, 96:128] = wfxh[h, D:DX]     # rows 160:256 stay zero
    wfxk16 = np.ascontiguousarray(
        wfxk.transpose(1, 0, 2).astype(NPBF16))  # [128, 24, 256]

    obias = (opb + opw @ (ipb[2 * D:3 * D] + rb[D:2 * D]))[None, :]
    obias = np.ascontiguousarray(np.repeat(obias.astype(f), SC, axis=0))
    rowmask = np.ascontiguousarray((~no_valid).astype(f)[:, None])

    in_maps = []
    for c in range(NCORES):
        sl = slice(c * SC, (c + 1) * SC)
        in_maps.append({
            "txt": np.ascontiguousarray(txt[c]),
            "txn": np.ascontiguousarray(txn[c]),
            "tx2": np.ascontiguousarray(tx2[c]),
            "qk01": np.ascontiguousarray(qk01[c]),
            "qk2": np.ascontiguousarray(qk2[c]),
            "wfx": wfxk16,
            "obias": obias,
            "rmask": rowmask[sl],
        })
    return in_maps


_NC_CACHE = {}


def get_nc(sc=SC):
    if sc not in _NC_CACHE:
        _NC_CACHE[sc] = build(sc)
    return _NC_CACHE[sc]


def run(in_maps, trace=False):
    nc = get_nc(SC)
    return run_bass_kernel_spmd(nc, in_maps, list(range(NCORES)), trace=trace)


def kernel(**inputs):
    in_maps = host_prep(**inputs)
    res = run(in_maps).results
    out = np.concatenate([res[c]["out"] for c in range(NCORES)], axis=0)
    return np.ascontiguousarray(out.reshape(B, S, D))
